# revision 1
# baseline (speedup 1.0000x reference)
"""CenterLoss Trainium2 kernel — raw Bacc + 4x indirect gather (v3).

Per core (512 samples, chunk = 128 samples):
  sync  : idx DMA -> x DMA -> (wait) -> out DMA
  gpsimd: 4x indirect_dma_start, one row per partition per chunk
  vector: per-chunk tensor_sub; final PSUM->SBUF reduce
  scalar: warm-up Square (hoists act-table load); per-chunk Square+accum
  tensor: ones.T @ d partition reduction -> PSUM [1,4]

Layouts:
  lab_t[p, n]  = labels[p*4 + n]   (chunk n, partition p)
  x_t[p, n, :] = x[p*4 + n, :]     (straight contiguous copy)
  c_t[p, n, :] = centers[lab_t[p, n]]
  d[p, n]      = ||x_t[p,n,:] - c_t[p,n,:]||^2
partial = sum_p sum_n d  ->  [1,1]
"""

import sys

import numpy as np

if "/opt/trn_rl_repo" not in sys.path:
    sys.path.insert(0, "/opt/trn_rl_repo")

B = 4096
D = 256
C = 8192
M = 8
SHARD = B // M   # 512
P = 128
NT = SHARD // P  # 4 chunks per core

_CACHE = {}


def build_nc():
    import concourse.bacc as bacc
    import concourse.bass as bass
    import concourse.mybir as mybir

    f32 = mybir.dt.float32
    i32 = mybir.dt.int32

    nc = bacc.Bacc("TRN2")
    x = nc.dram_tensor("x", [SHARD, D], f32, kind="ExternalInput")
    lab = nc.dram_tensor("lab", [P, NT], i32, kind="ExternalInput")
    cen = nc.dram_tensor("cen", [C, D], f32, kind="ExternalInput")
    out = nc.dram_tensor("out", [1, 1], f32, kind="ExternalOutput")

    ones = nc.const_aps.aps[(f32, 1.0)]  # [128, 1], set up in Bass preamble

    with (
        nc.sbuf_tensor("x_t", [P, NT, D], f32) as x_t,
        nc.sbuf_tensor("c_t", [P, NT, D], f32) as c_t,
        nc.sbuf_tensor("diff", [P, NT, D], f32) as diff,
        nc.sbuf_tensor("sq", [P, NT, D], f32) as sq,
        nc.sbuf_tensor("lab_t", [P, NT], i32) as lab_t,
        nc.sbuf_tensor("d", [P, NT], f32) as d,
        nc.sbuf_tensor("res", [1, 1], f32) as res,
        nc.sbuf_tensor("warm", [1, 1], f32) as warm,
        nc.psum_tensor([1, NT], f32) as ps,
        nc.semaphore("i_s") as i_s,
        nc.semaphore("x_s") as x_s,
        nc.semaphore("g0_s") as g0_s,
        nc.semaphore("g1_s") as g1_s,
        nc.semaphore("g2_s") as g2_s,
        nc.semaphore("g3_s") as g3_s,
        nc.semaphore("v_s") as v_s,
        nc.semaphore("a_s") as a_s,
        nc.semaphore("t_s") as t_s,
        nc.semaphore("o_s") as o_s,
        nc.Block() as block,
    ):
        g_sems = (g0_s, g1_s, g2_s, g3_s)

        hoist = []

        @block.sync
        def _(sync):
            hoist.append(
                sync.dma_start(lab_t[:, :], lab[:, :]).then_inc(i_s, 16)
            )
            sync.dma_start(
                x_t[:, :, :], x[:, :].rearrange("(p n) d -> p n d", p=P)
            ).then_inc(x_s, 16)
            sync.wait_ge(v_s, NT + 1)
            hoist.append(
                sync.dma_start(out[:, :], res[:, :]).then_inc(o_s, 16)
            )

        @block.gpsimd
        def _(g):
            g.wait_ge(i_s, 16)
            for n, gs in enumerate(g_sems):
                g.indirect_dma_start(
                    out=c_t[:, n, :],
                    out_offset=None,
                    in_=cen[:, :],
                    in_offset=bass.IndirectOffsetOnAxis(
                        ap=lab_t[:, n : n + 1], axis=0
                    ),
                ).then_inc(gs, 16)

        @block.vector
        def _(v):
            v.wait_ge(x_s, 16)
            for n, gs in enumerate(g_sems):
                v.wait_ge(gs, 16)
                v.tensor_sub(
                    diff[:, n, :], x_t[:, n, :], c_t[:, n, :]
                ).then_inc(v_s, 1)
            v.wait_ge(t_s, 1)
            v.reduce_sum(
                res[:, :], ps[:, :], axis=mybir.AxisListType.X
            ).then_inc(v_s, 1)

        @block.scalar
        def _(s):
            # dummy op forces the Square act-table load at ACT program
            # start, off the critical path
            s.activation(
                warm[:, :], ones[:1, :], mybir.ActivationFunctionType.Square
            )
            for n in range(NT):
                s.wait_ge(v_s, n + 1)
                s.activation(
                    sq[:, n, :],
                    diff[:, n, :],
                    mybir.ActivationFunctionType.Square,
                    accum_out=d[:, n : n + 1],
                ).then_inc(a_s, 1)

        @block.tensor
        def _(t):
            t.wait_ge(a_s, NT)
            t.matmul(
                ps[:, :], lhsT=ones, rhs=d[:, :], start=True, stop=True
            ).then_inc(t_s, 1)

    # hoist the label DMA into the entry block, after SP's drain but
    # before SP's barrier-arrival EVSEM: the DMA then issues during the
    # const-init barrier and its ~1.7us completion receipt overlaps it
    entry = nc.m.functions[0].blocks[0]
    lab_inst = hoist[0].ins
    for blk in nc.m.functions[0].blocks:
        if lab_inst in blk.instructions:
            blk.instructions.remove(lab_inst)
            break
    sp_barrier_idx = next(
        i
        for i, ins in enumerate(entry.instructions)
        if ins.name.startswith("barrier_SP")
    )
    entry.instructions.insert(sp_barrier_idx, lab_inst)

    # End-block restructure for SP: its DRAIN blocks on the out-DMA
    # completion receipt and carries the barrier-arrival inc, so all
    # engines' teardown waits ~1.7us for the receipt. Move the arrival
    # inc to a fresh EVSEM at the drain's old slot and run the drain
    # after the barrier passes - every engine still drains its own DMAs
    # before its stream ends, but the barrier releases early.
    end_blk = nc.m.functions[0].blocks[-1]
    sp_drain = next(
        ins
        for ins in end_blk.instructions
        if isinstance(ins, mybir.InstDrain)
        and ins.engine == mybir.EngineType.SP
    )
    sp_evsem = next(
        ins for ins in end_blk.instructions if ins.name.startswith("barrier_SP")
    )
    arrive = mybir.InstEventSemaphore(
        name=nc.get_next_instruction_name(), ins=[], outs=[]
    )
    arrive.engine = mybir.EngineType.SP
    arrive.sync_info = sp_drain.sync_info
    sp_drain.sync_info = None
    nc.register_instruction(arrive)
    end_blk.instructions.remove(sp_drain)
    ei = end_blk.instructions.index(sp_evsem)
    end_blk.instructions.insert(ei + 1, sp_drain)
    # place the arrival in SP's body BEFORE the out-DMA issue: the
    # barrier release then overlaps the 0.64us issue slice instead of
    # trailing it
    out_inst = hoist[1].ins
    body_blk = next(
        blk
        for blk in nc.m.functions[0].blocks
        if out_inst in blk.instructions
    )
    oi = body_blk.instructions.index(out_inst)
    body_blk.instructions.insert(oi, arrive)

    nc.compile()
    return nc


def _get_nc():
    if "nc" not in _CACHE:
        _CACHE["nc"] = build_nc()
    return _CACHE["nc"]


def make_in_maps(x, labels, centers):
    x = np.ascontiguousarray(np.asarray(x), dtype=np.float32)
    labels = np.ascontiguousarray(np.asarray(labels)).astype(np.int32)
    centers = np.ascontiguousarray(np.asarray(centers), dtype=np.float32)
    in_maps = []
    for i in range(M):
        ls = labels[i * SHARD : (i + 1) * SHARD]
        in_maps.append(
            {
                "x": x[i * SHARD : (i + 1) * SHARD],
                # lab_t[p, n] = labels[p*4 + n]
                "lab": np.ascontiguousarray(ls.reshape(P, NT)),
                "cen": centers,
            }
        )
    return in_maps


def finish(partials):
    total = float(np.sum(np.asarray(partials, dtype=np.float64)))
    total += B * (C - 1) * 1e-12  # masked-out entries clamp to 1e-12
    return np.float32(total / B)


def kernel(x, labels, centers):
    from concourse import bass_utils

    nc = _get_nc()
    res = bass_utils.run_bass_kernel_spmd(
        nc, make_in_maps(x, labels, centers), list(range(M))
    )
    return finish([r["out"][0, 0] for r in res.results])

